# revision 9
# baseline (speedup 1.0000x reference)
"""3-layer GCN (GCNConv+BN+ReLU x2, GCNConv, softmax) on 8 Trainium2 NeuronCores.

Strategy (graph/data parallel, per sharding hint):
  - Nodes sharded 6250/core. Edges partitioned by destination core, sorted by
    dst, grouped into 128-dst windows, padded to 128-edge tiles.
  - Symmetric normalization coef = dinv[src]*dinv[dst] is factored into
    per-node pre/post scaling (dinv), folded into the activation stages, so
    per-edge work is a pure row gather + one-hot matmul scatter-add.
  - BatchNorm (eval) folded into the conv weights/bias on the host.
  - Per layer: H_local = act_local @ W (TensorE), AllGather H (collective),
    then per dst-window: dma_gather rows of H, build one-hot P via iota
    is_equal (VectorE), accumulate P.T @ msg into PSUM (TensorE),
    postprocess (bias/relu/dinv scaling).
  - Gathers use int16 indices, so sources are bucketed lo/hi around
    SPLIT=25000 (table viewed at two offsets). Tiles of each bucket form two
    global streams fetched in 8-tile chunks (1024 indices/call — the HW
    limit for one SWDGE gather).
  - Final layer: width-64 aggregation + softmax, output f32 shards.
"""
import sys
import time

sys.path.insert(0, "/opt/trn_rl_repo")

import numpy as np
import ml_dtypes

from concourse import bass, mybir, bacc, tile
from concourse import bass_utils

# problem constants (hardcoded per contract)
N, E = 50000, 800000
IN = 256
HID = 256
OUT = 64
OUTP = 128  # padded width for layer-3 H (dma_gather needs >=256B rows)
C = 8
NS = N // C  # 6250 nodes per core
P = 128
NW = (NS + P - 1) // P  # 49 windows per core
SPLIT = 25000  # src-index split so indices fit int16
EPS = 1e-5
CHUNK = 8  # tiles per dma_gather call (8*128 = 1024 idx = HW limit)

BF = mybir.dt.bfloat16
F32 = mybir.dt.float32
I16 = mybir.dt.int16

AX = mybir.AluOpType
AF = mybir.ActivationFunctionType


# --------------------------------------------------------------------------
# host-side preprocessing
# --------------------------------------------------------------------------
def _host_prep(inputs):
    x = np.asarray(inputs["x"], np.float32)
    ei = np.asarray(inputs["edge_index"]).astype(np.int64)
    W1 = np.asarray(inputs["W1"], np.float32)
    b1 = np.asarray(inputs["b1"], np.float32)
    g1 = np.asarray(inputs["g1"], np.float32)
    beta1 = np.asarray(inputs["beta1"], np.float32)
    m1 = np.asarray(inputs["m1"], np.float32)
    v1 = np.asarray(inputs["v1"], np.float32)
    W2 = np.asarray(inputs["W2"], np.float32)
    b2 = np.asarray(inputs["b2"], np.float32)
    g2 = np.asarray(inputs["g2"], np.float32)
    beta2 = np.asarray(inputs["beta2"], np.float32)
    m2 = np.asarray(inputs["m2"], np.float32)
    v2 = np.asarray(inputs["v2"], np.float32)
    W3 = np.asarray(inputs["W3"], np.float32)
    b3 = np.asarray(inputs["b3"], np.float32)

    # self loops
    loops = np.arange(N, dtype=np.int64)
    src = np.concatenate([ei[0], loops])
    dst = np.concatenate([ei[1], loops])

    deg = np.bincount(dst, minlength=N).astype(np.float32)
    dinv = (1.0 / np.sqrt(np.maximum(deg, 1.0))).astype(np.float32)

    # fold BN into conv weights/bias
    s1 = g1 / np.sqrt(v1 + EPS)
    W1e = W1 * s1[None, :]
    c1 = (b1 - m1) * s1 + beta1
    s2 = g2 / np.sqrt(v2 + EPS)
    W2e = W2 * s2[None, :]
    c2 = (b2 - m2) * s2 + beta2
    W3e = np.concatenate([W3, np.zeros((HID, OUTP - OUT), np.float32)], axis=1)
    c3 = b3

    # edge grouping: (owner core, window, lo/hi src bucket)
    owner = dst // NS
    wl = (dst % NS) // P
    bkt = (src >= SPLIT).astype(np.int64)
    grp = (owner * NW + wl) * 2 + bkt
    order = np.argsort(grp, kind="stable")
    g = grp[order]
    ss = src[order]
    dd = dst[order]

    cnt = np.bincount(g, minlength=C * NW * 2)
    cnt3 = cnt.reshape(C, NW, 2)
    KA = np.ceil(cnt3[:, :, 0].max(axis=0) / P).astype(np.int64)  # [NW]
    KB = np.ceil(cnt3[:, :, 1].max(axis=0) / P).astype(np.int64)
    K = KA + KB
    # window-contiguous tile order (for loc / P / matmuls)
    tile_base = np.concatenate([[0], np.cumsum(K)]).astype(np.int64)
    NTT = int(K.sum())
    # bucket stream order (for gather chunks)
    lo_base = np.concatenate([[0], np.cumsum(KA)]).astype(np.int64)
    hi_base = np.concatenate([[0], np.cumsum(KB)]).astype(np.int64)
    NLO = int(KA.sum())
    NHI = int(KB.sum())
    NLOp = (NLO + CHUNK - 1) // CHUNK * CHUNK
    NHIp = (NHI + CHUNK - 1) // CHUNK * CHUNK

    starts = np.concatenate([[0], np.cumsum(cnt)]).astype(np.int64)
    rank = np.arange(g.size, dtype=np.int64) - starts[g]
    ow = g >> 1
    c_of = ow // NW
    w_of = ow % NW
    b_of = g & 1
    # stream slot (gather order): position within lo/hi stream
    sslot = np.where(
        b_of == 0, lo_base[w_of] + rank // P, hi_base[w_of] + rank // P
    )
    # window slot (matmul/P order)
    wslot = tile_base[w_of] + np.where(b_of == 0, 0, KA[w_of]) + rank // P
    part = rank % P

    idx16 = np.zeros((C, P, 8 * (NLOp + NHIp)), np.int16)
    locv = np.full((C, P, NTT, 1), -1.0, np.float32)
    for c in range(C):
        m = c_of == c
        val = (ss[m] - np.where(b_of[m] == 1, SPLIT, 0)).astype(np.int16)
        mlo = m & (b_of == 0)
        mhi = m & (b_of == 1)
        seq_lo = np.zeros(NLOp * P, np.int16)
        seq_lo[sslot[mlo] * P + part[mlo]] = (ss[mlo]).astype(np.int16)
        seq_hi = np.zeros(NHIp * P, np.int16)
        seq_hi[sslot[mhi] * P + part[mhi]] = (ss[mhi] - SPLIT).astype(np.int16)
        seq = np.concatenate([seq_lo, seq_hi])
        # 16-partition wrap, replicated 8x: element i -> [i % 16, i // 16]
        idx16[c] = np.tile(seq.reshape(-1, 16).T, (8, 1))
        locv[c, part[m], wslot[m], 0] = (dd[m] - c * NS - w_of[m] * P).astype(
            np.float32
        )

    # per-core dinv layout [P, NW]: node c*NS + w*128 + p, padded with 1.0
    dinv_sb = np.ones((C, P, NW), np.float32)
    for c in range(C):
        dc = dinv[c * NS : (c + 1) * NS]
        dpad = np.concatenate([dc, np.ones(NW * P - NS, np.float32)])
        dinv_sb[c] = dpad.reshape(NW, P).T

    iota = np.zeros((P, 1, P), np.float32)
    iota[:, 0, :] = np.arange(P)[None, :]
    ident = np.eye(P, dtype=np.float32)

    shared = {
        "w1": W1e.astype(ml_dtypes.bfloat16),
        "w2": W2e.astype(ml_dtypes.bfloat16),
        "w3": W3e.astype(ml_dtypes.bfloat16),
        "b1r": np.tile(c1, (P, 1)).astype(np.float32),
        "b2r": np.tile(c2, (P, 1)).astype(np.float32),
        "b3r": np.tile(c3, (P, 1)).astype(np.float32),
        "iota": iota.astype(ml_dtypes.bfloat16),
        "ident": ident.astype(ml_dtypes.bfloat16),
    }
    in_maps = []
    for c in range(C):
        m = dict(shared)
        m["xs"] = np.ascontiguousarray(x[c * NS : (c + 1) * NS])
        m["idx16"] = np.ascontiguousarray(idx16[c])
        m["loc"] = np.ascontiguousarray(locv[c].astype(ml_dtypes.bfloat16))
        m["dinv"] = np.ascontiguousarray(dinv_sb[c])
        in_maps.append(m)

    meta = dict(
        KA=KA.tolist(),
        KB=KB.tolist(),
        tile_base=tile_base.tolist(),
        lo_base=lo_base.tolist(),
        hi_base=hi_base.tolist(),
        NTT=NTT,
        NLOp=NLOp,
        NHIp=NHIp,
    )
    return in_maps, meta


# --------------------------------------------------------------------------
# device program
# --------------------------------------------------------------------------
def _build(meta, reps=1):
    KA, KB = meta["KA"], meta["KB"]
    tile_base, lo_base, hi_base = meta["tile_base"], meta["lo_base"], meta["hi_base"]
    NTT, NLOp, NHIp = meta["NTT"], meta["NLOp"], meta["NHIp"]
    NIDX = 8 * (NLOp + NHIp)

    nc = bacc.Bacc("TRN2", target_bir_lowering=False, debug=False, num_devices=C)

    xs = nc.dram_tensor("xs", [NS, IN], F32, kind="ExternalInput")
    idx16 = nc.dram_tensor("idx16", [P, NIDX], I16, kind="ExternalInput")
    locd = nc.dram_tensor("loc", [P, NTT, 1], BF, kind="ExternalInput")
    dinvd = nc.dram_tensor("dinv", [P, NW], F32, kind="ExternalInput")
    w1d = nc.dram_tensor("w1", [IN, HID], BF, kind="ExternalInput")
    w2d = nc.dram_tensor("w2", [HID, HID], BF, kind="ExternalInput")
    w3d = nc.dram_tensor("w3", [HID, OUTP], BF, kind="ExternalInput")
    b1d = nc.dram_tensor("b1r", [P, HID], F32, kind="ExternalInput")
    b2d = nc.dram_tensor("b2r", [P, HID], F32, kind="ExternalInput")
    b3d = nc.dram_tensor("b3r", [P, OUT], F32, kind="ExternalInput")
    iotad = nc.dram_tensor("iota", [P, 1, P], BF, kind="ExternalInput")
    identd = nc.dram_tensor("ident", [P, P], BF, kind="ExternalInput")
    outd = nc.dram_tensor("out", [NS, OUT], F32, kind="ExternalOutput")

    with tile.TileContext(nc) as tc:
        with (
            tc.tile_pool(name="const", bufs=1) as cp,
            tc.tile_pool(name="dram", bufs=1, space="DRAM") as dp,
            tc.tile_pool(name="work", bufs=3) as wp,
            tc.tile_pool(name="mlo", bufs=6) as mplo,
            tc.tile_pool(name="mhi", bufs=6) as mphi,
            tc.tile_pool(name="pwp", bufs=4) as pp,
            tc.tile_pool(name="smax", bufs=3) as sp,
            tc.tile_pool(name="ps_a", bufs=2, space="PSUM") as ps_a,
            tc.tile_pool(name="ps_h", bufs=2, space="PSUM") as ps_h,
            tc.tile_pool(name="ps_t", bufs=2, space="PSUM") as ps_t,
        ):
            # ---- persistent constants in SBUF
            idx_sb = cp.tile([P, NIDX], I16, name="idx_sb", tag="idx_sb")
            nc.sync.dma_start(out=idx_sb[:], in_=idx16[:])
            loc_sb = cp.tile([P, NTT, 1], BF, name="loc_sb", tag="loc_sb")
            nc.sync.dma_start(out=loc_sb[:], in_=locd[:])
            dinv_sb = cp.tile([P, NW], F32, name="dinv_sb", tag="dinv_sb")
            nc.sync.dma_start(out=dinv_sb[:], in_=dinvd[:])
            iota_sb = cp.tile([P, 1, P], BF, name="iota_sb", tag="iota_sb")
            nc.sync.dma_start(out=iota_sb[:], in_=iotad[:])
            ident_sb = cp.tile([P, P], BF, name="ident_sb", tag="ident_sb")
            nc.sync.dma_start(out=ident_sb[:], in_=identd[:])

            w_sb = {}
            for nm, dt_, dout in (("w1", w1d, HID), ("w2", w2d, HID), ("w3", w3d, OUTP)):
                t = cp.tile([P, 2 * dout], BF, name=f"{nm}_sb", tag=f"{nm}_sb")
                for kb in range(2):
                    nc.sync.dma_start(
                        out=t[:, kb * dout : (kb + 1) * dout],
                        in_=dt_[kb * P : (kb + 1) * P, :],
                    )
                w_sb[nm] = t
            b_sb = {}
            for nm, dt_, dout in (("b1", b1d, HID), ("b2", b2d, HID), ("b3", b3d, OUT)):
                t = cp.tile([P, dout], F32, name=f"{nm}_sb", tag=f"{nm}_sb")
                nc.sync.dma_start(out=t[:], in_=dt_[:])
                b_sb[nm] = t

            # ---- internal DRAM (per rep: a Shared tensor allows only 1 writer)
            h_loc = {}
            h_full = {}
            for r in range(reps):
                for nm0, d in (("h1", HID), ("h2", HID), ("h3", OUTP)):
                    nm = f"{nm0}_{r}"
                    h_loc[nm] = dp.tile(
                        [NS, d], BF, name=f"{nm}_loc", tag=f"{nm}_loc"
                    )
                    h_full[nm] = dp.tile(
                        [N, d], BF, name=f"{nm}_full", tag=f"{nm}_full",
                        addr_space="Shared",
                    )

            def h_stage(m, rows, act_ap, w_t, dout, h_loc_t):
                """act tile [P, 256] bf16 (node-major) -> H tile -> h_loc rows."""
                aTs = []
                for kb in range(2):
                    pt = ps_t.tile([P, P], BF, name=f"pt{kb}", tag=f"pt{kb}")
                    nc.tensor.transpose(
                        out=pt[:],
                        in_=act_ap[:, kb * P : (kb + 1) * P],
                        identity=ident_sb[:],
                    )
                    aT = wp.tile([P, P], BF, name=f"aT{kb}", tag=f"aT{kb}")
                    nc.scalar.copy(out=aT[:], in_=pt[:])
                    aTs.append(aT)
                ph = ps_h.tile([P, dout], F32, name="ph", tag="ph")
                for kb in range(2):
                    nc.tensor.matmul(
                        out=ph[:],
                        lhsT=aTs[kb][:],
                        rhs=w_t[:, kb * dout : (kb + 1) * dout],
                        start=(kb == 0),
                        stop=(kb == 1),
                    )
                h_t = wp.tile([P, dout], BF, name="h_t", tag="h_t")
                nc.vector.tensor_copy(out=h_t[:], in_=ph[:])
                nc.sync.dma_start(
                    out=h_loc_t[m * P : m * P + rows, :], in_=h_t[:rows, :]
                )

            def allgather(nm):
                nc.gpsimd.collective_compute(
                    "AllGather",
                    AX.bypass,
                    replica_groups=[list(range(C))],
                    ins=[h_loc[nm][:].opt()],
                    outs=[h_full[nm][:].opt()],
                )

            def agg_stage(h_full_t, elem, dagg, bias_t, w_next, dnext, h_next_t, last):
                lo_ch = {}
                hi_ch = {}
                issued = {"lo": 0, "hi": 0}

                def issue(stream, cid):
                    pool = mplo if stream == "lo" else mphi
                    base_col = 0 if stream == "lo" else 8 * NLOp
                    t = pool.tile(
                        [P, CHUNK, elem], BF, name=f"m{stream}", tag=f"m{stream}"
                    )
                    nc.gpsimd.dma_gather(
                        out_ap=t[:, :, :],
                        in_ap=h_full_t[:SPLIT, :] if stream == "lo" else h_full_t[SPLIT:, :],
                        idxs_ap=idx_sb[
                            :, base_col + 8 * CHUNK * cid : base_col + 8 * CHUNK * (cid + 1)
                        ],
                        num_idxs=CHUNK * P,
                        num_idxs_reg=CHUNK * P,
                        elem_size=elem,
                    )
                    (lo_ch if stream == "lo" else hi_ch)[cid] = t

                for w in range(NW):
                    rows = min(P, NS - w * P)
                    tb = int(tile_base[w])
                    ka = int(KA[w])
                    kb_ = int(KB[w])
                    k = ka + kb_
                    if ka:
                        need = (int(lo_base[w]) + ka - 1) // CHUNK
                        while issued["lo"] <= need:
                            issue("lo", issued["lo"])
                            issued["lo"] += 1
                    if kb_:
                        need = (int(hi_base[w]) + kb_ - 1) // CHUNK
                        while issued["hi"] <= need:
                            issue("hi", issued["hi"])
                            issued["hi"] += 1
                    Pw = pp.tile([P, k, P], BF, name="Pw", tag="Pw")
                    nc.vector.tensor_tensor(
                        out=Pw[:],
                        in0=loc_sb[:, tb : tb + k, :1].to_broadcast([P, k, P]),
                        in1=iota_sb[:].to_broadcast([P, k, P]),
                        op=AX.is_equal,
                    )
                    pa = ps_a.tile([P, dagg], F32, name="pa", tag="pa")
                    for kk in range(k):
                        if kk < ka:
                            sid = int(lo_base[w]) + kk
                            t = lo_ch[sid // CHUNK]
                        else:
                            sid = int(hi_base[w]) + (kk - ka)
                            t = hi_ch[sid // CHUNK]
                        nc.tensor.matmul(
                            out=pa[:],
                            lhsT=Pw[:, kk, :],
                            rhs=t[:, sid % CHUNK, :dagg],
                            start=(kk == 0),
                            stop=(kk == k - 1),
                        )
                    if not last:
                        t1 = wp.tile([P, dagg], F32, name="t1", tag="t1")
                        nc.vector.scalar_tensor_tensor(
                            out=t1[:],
                            in0=pa[:],
                            scalar=dinv_sb[:, w : w + 1],
                            in1=bias_t[:],
                            op0=AX.mult,
                            op1=AX.add,
                        )
                        act_t = wp.tile([P, dagg], BF, name="act_t", tag="act_t")
                        nc.scalar.activation(
                            out=act_t[:],
                            in_=t1[:],
                            func=AF.Relu,
                            scale=dinv_sb[:, w : w + 1],
                        )
                        h_stage(w, rows, act_t[:], w_next, dnext, h_next_t)
                    else:
                        t1 = sp.tile([P, OUT], F32, name="t1s", tag="t1s")
                        nc.vector.scalar_tensor_tensor(
                            out=t1[:],
                            in0=pa[:],
                            scalar=dinv_sb[:, w : w + 1],
                            in1=bias_t[:],
                            op0=AX.mult,
                            op1=AX.add,
                        )
                        negm = sp.tile([P, 1], F32, name="negm", tag="negm")
                        nc.vector.tensor_reduce(
                            out=negm[:],
                            in_=t1[:],
                            axis=mybir.AxisListType.X,
                            op=AX.max,
                            negate=True,
                        )
                        ex = sp.tile([P, OUT], F32, name="ex", tag="ex")
                        ssum = sp.tile([P, 1], F32, name="ssum", tag="ssum")
                        nc.scalar.activation(
                            out=ex[:],
                            in_=t1[:],
                            func=AF.Exp,
                            bias=negm[:],
                            accum_out=ssum[:],
                        )
                        rinv = sp.tile([P, 1], F32, name="rinv", tag="rinv")
                        nc.vector.reciprocal(out=rinv[:], in_=ssum[:])
                        o = sp.tile([P, OUT], F32, name="o", tag="o")
                        nc.vector.tensor_scalar_mul(out=o[:], in0=ex[:], scalar1=rinv[:])
                        nc.sync.dma_start(
                            out=outd[w * P : w * P + rows, :], in_=o[:rows, :]
                        )

            for r in range(reps):
                h1, h2, h3 = f"h1_{r}", f"h2_{r}", f"h3_{r}"
                # stage A: act0 = dinv * x -> H1 -> h1_loc
                for m in range(NW):
                    rows = min(P, NS - m * P)
                    x_t = wp.tile([P, IN], F32, name="x_t", tag="x_t")
                    if rows < P:
                        nc.vector.memset(x_t[:], 0.0)
                    nc.sync.dma_start(
                        out=x_t[:rows, :], in_=xs[m * P : m * P + rows, :]
                    )
                    act_t = wp.tile([P, IN], BF, name="act_t", tag="act_t")
                    nc.vector.tensor_scalar_mul(
                        out=act_t[:], in0=x_t[:], scalar1=dinv_sb[:, m : m + 1]
                    )
                    h_stage(m, rows, act_t[:], w_sb["w1"], HID, h_loc[h1])
                allgather(h1)
                # stage B: agg layer1 -> act1 -> H2 -> h2_loc
                agg_stage(
                    h_full[h1], HID, HID, b_sb["b1"], w_sb["w2"], HID,
                    h_loc[h2], last=False,
                )
                allgather(h2)
                # stage C: agg layer2 -> act2 -> H3 -> h3_loc
                agg_stage(
                    h_full[h2], HID, HID, b_sb["b2"], w_sb["w3"], OUTP,
                    h_loc[h3], last=False,
                )
                allgather(h3)
                # stage D: agg layer3 (width 64) + softmax -> out
                agg_stage(
                    h_full[h3], OUTP, OUT, b_sb["b3"], None, 0, None, last=True
                )

    nc.compile()
    return nc


# --------------------------------------------------------------------------
# persistent-staging runner (inputs stay device-resident between calls)
# --------------------------------------------------------------------------
def _make_runner(nc, in_maps):
    import jax
    from jax.experimental.shard_map import shard_map
    from jax.sharding import Mesh, NamedSharding, PartitionSpec

    from concourse import bass2jax, mybir as mb

    bass2jax.install_neuronx_cc_hook()

    in_names, out_names, out_avals, zero_shapes = [], [], [], []
    for alloc in nc.m.functions[0].allocations:
        if not isinstance(alloc, mb.MemoryLocationSet):
            continue
        name = alloc.memorylocations[0].name
        if alloc.kind == "ExternalInput":
            in_names.append(name)
        elif alloc.kind == "ExternalOutput":
            out_names.append(name)
            shape = tuple(alloc.tensor_shape)
            dtype = mb.dt.np(alloc.dtype)
            out_avals.append(jax.core.ShapedArray(shape, dtype))
            zero_shapes.append((shape, dtype))
    part_name = nc.partition_id_tensor.name if nc.partition_id_tensor else None
    if part_name is not None and part_name in in_names:
        in_names.remove(part_name)
    n_params = len(in_names)
    n_outs = len(out_names)
    all_names = in_names + out_names + ([part_name] if part_name else [])

    def _body(*args):
        operands = list(args)
        if part_name is not None:
            operands.append(bass2jax.partition_id_tensor())
        outs = bass2jax._bass_exec_p.bind(
            *operands,
            out_avals=tuple(out_avals),
            in_names=tuple(all_names),
            out_names=tuple(out_names),
            lowering_input_output_aliases=(),
            sim_require_finite=True,
            sim_require_nnan=True,
            nc=nc,
        )
        return tuple(outs)

    devices = jax.devices()[:C]
    mesh = Mesh(np.asarray(devices), ("core",))
    in_specs = (PartitionSpec("core"),) * (n_params + n_outs)
    out_specs = (PartitionSpec("core"),) * n_outs
    donate = tuple(range(n_params, n_params + n_outs))
    sharded = jax.jit(
        shard_map(_body, mesh=mesh, in_specs=in_specs, out_specs=out_specs,
                  check_rep=False),
        donate_argnums=donate,
        keep_unused=True,
    )
    sh = NamedSharding(mesh, PartitionSpec("core"))
    in_dev = [
        jax.device_put(
            np.concatenate([np.asarray(in_maps[c][n]) for c in range(C)], axis=0), sh
        )
        for n in in_names
    ]

    def run():
        zeros = [
            jax.device_put(np.zeros((C * s[0], *s[1:]), d), sh)
            for s, d in zero_shapes
        ]
        outs = sharded(*in_dev, *zeros)
        jax.block_until_ready(outs)
        return [
            {
                n: np.asarray(outs[i]).reshape(C, *out_avals[i].shape)[c]
                for i, n in enumerate(out_names)
            }
            for c in range(C)
        ]

    return run


# --------------------------------------------------------------------------
# entry points
# --------------------------------------------------------------------------
def _execute(inputs, reps=1, runs=1):
    in_maps, meta = _host_prep(inputs)
    t0 = time.time()
    nc = _build(meta, reps=reps)
    t1 = time.time()
    walls = []
    res = None
    for _ in range(runs):
        ts = time.time()
        res = bass_utils.run_bass_kernel_spmd(nc, in_maps, list(range(C)))
        walls.append(time.time() - ts)
    out = np.concatenate([res.results[c]["out"] for c in range(C)], axis=0)
    return out, dict(build_s=t1 - t0, walls=walls)


def kernel(**inputs) -> np.ndarray:
    out, _ = _execute(inputs, reps=1, runs=1)
    return out.astype(np.float32)


if __name__ == "__main__":
    rng = np.random.default_rng(0)
    d = {
        "x": rng.standard_normal((N, IN)).astype(np.float32),
        "edge_index": rng.integers(0, N, size=(2, E)).astype(np.int32),
    }
    for i, (di, do) in enumerate(((IN, HID), (HID, HID), (HID, OUT)), 1):
        d[f"W{i}"] = (rng.standard_normal((di, do)) * 0.05).astype(np.float32)
        d[f"b{i}"] = np.zeros(do, np.float32)
        if i < 3:
            d[f"g{i}"] = np.ones(do, np.float32)
            d[f"beta{i}"] = np.zeros(do, np.float32)
            d[f"m{i}"] = (rng.standard_normal(do) * 0.1).astype(np.float32)
            d[f"v{i}"] = rng.uniform(0.5, 1.5, do).astype(np.float32)
    out, info = _execute(d)
    print("out shape:", out.shape, "info:", info)
